# revision 24
# baseline (speedup 1.0000x reference)
"""Differential attention kernel for Trainium2 (8 NeuronCores, SPMD).

Math per (batch, head):
    q1,q2 / k1,k2 = halves of head_dim (D=64 -> d2=32)
    a_i = softmax(q_i @ k_i^T / sqrt(d2))        (i = 1,2)
    out = (a1 - lam*a2) @ V, then per-(q) groupnorm over D, scaled by (1-0.8).

Design (per core: 4 of the 32 (b,h) pairs). ~34M exp evals/core dominate;
they are split across TWO engines: ScalarE ACTIVATE(Exp) drains the 4-bank
PSUM score tiles, and the DVE drains four of the six 2-bank tiles with a
one-instruction Schraudolph fast-exp: u16 = int16(s*A + B) bit-viewed as
fp16 (A = 1024*log2e*scale, B = 15*1024-45; |rel err| <= 3%, applied to
25% of scores; final output err ~1.4e-2 < 2e-2 tolerance, sim-verified).
  - Q/K/V cast to fp16 on host; Q^T/K^T shipped twice ([2D, s]) so 4 score
    matmuls (K=32 contraction) run concurrently in the four 32-row PE row
    groups via tile_position.
  - Scores transposed: S^T[k, q] units [128, 512] packed in alternating
    4-bank (ScalarE) / 2-bank PSUM tiles; 2 remaining banks double-buffer
    the AV accumulators. exp needs no max-subtraction: scores ~ N(0,1).
  - U^T fp16 in SBUF; AV lhsT = [0 | V0..61 | 0 | V62 V63 | ones] (M=67).
    The dummy columns at positions 0 and 63 absorb a hardware hazard
    observed under ACT+DVE+PE concurrent PSUM access: AV-matmul outputs at
    PSUM partitions 0 and 63 are intermittently corrupted, so no real data
    lives there. The ones column accumulates softmax row-sums for free.
  - O^T[67, q] per 512-q chunk, fp16 to SBUF (padded to 80 rows), DMA xbar
    transpose to natural layout.
  - Norm path batched per chunk: 2 strided reciprocals + 1 scale + 6
    tensor_tensor (W = O1*r1inv - lam*O2*r2inv via step-0 broadcast APs,
    in a 62-wide and a 2-wide piece around the dummy row) + 4 bn_stats +
    4 bn_aggr.
  - rstd = (1-lam0)*rsqrt(var+eps) via Schraudolph ln seed + fp16
    Schraudolph exp + 2 Newton steps, dribbled between score groups.
"""

import math
import numpy as np

import concourse.bass as bass
import concourse.tile as tile
from concourse import bacc, mybir
from concourse.bass_utils import run_bass_kernel_spmd

F32 = mybir.dt.float32
F16 = mybir.dt.float16
I16 = mybir.dt.int16
I32 = mybir.dt.int32
AF = mybir.ActivationFunctionType
ALU = mybir.AluOpType

B, H, S, D = 2, 16, 2048, 64
D2 = D // 2
N_CORES = 8
HEADS_PER_CORE = (B * H) // N_CORES  # 4
LAMBDA_INIT = 0.8
EPS = 1e-5
SCALE = 1.0 / math.sqrt(D2)
LOG2E = 1.4426950408889634
LN2 = 0.6931471805599453

QC = 512           # q chunk (one PSUM bank of fp32)
KTILE = 128        # k tile (partition dim)
MAV = 67           # AV out rows: [dummy, V0..61, dummy, V62, V63, ones]
RROW = 66          # row-sum (ones) position in O^T
OROWS = 80         # O^T rows padded to xbar 16-row granularity

# Schraudolph fast-exp constants (int16 bits of fp16): round(t*EXP_A + EXP_B)
EXP_C = 45.0
EXP_A = 1024.0 * LOG2E * SCALE
EXP_B = 15.0 * 1024.0 - EXP_C
# rsqrt seed: ln(x) ~ (bits(x)*2^-23 - 127 + 0.043)*ln2;  seed = exp(-ln(x)/2)
RS_K1 = -0.5 * LN2 / (1 << 23)
RS_K2 = 0.5 * LN2 * (127.0 - 0.043)
RS_A = 1024.0 * LOG2E
RS_B = 15.0 * 1024.0 - EXP_C

DVE_SPLIT = True  # slot 3 of each 4-bank tile exp'd on the DVE (f=0.1875)


def _group_sizes(nunits):
    """Score units per PSUM tile: [4,2]*4 + [4,4]. The 4-bank (ps4) and
    2-bank (ps2) tiles alternate so ScalarE ping-pongs between two buffers
    (the PE refills one while ScalarE drains the other). Slot 3 of each
    4-bank tile is drained by the DVE instead (Schraudolph)."""
    assert nunits == 32
    return [4, 2, 4, 2, 4, 2, 4, 2, 4, 4]


def build_program(n_heads=HEADS_PER_CORE, s=S, u_bufs=26):
    nq = s // QC          # q chunks per head
    nkt = s // KTILE      # k tiles per head
    nqt = s // 128        # q tiles (norm phase)
    tpq = QC // 128       # q tiles per chunk
    nunits = 2 * nkt      # score units per q chunk (2 halves x k tiles)
    gsizes = _group_sizes(nunits)
    gstarts = [sum(gsizes[:i]) for i in range(len(gsizes))]
    ngrp = len(gsizes)

    def unit_slot(j):
        """U-tile index and within-tile slot for score unit j."""
        for g, (st0, sz) in enumerate(zip(gstarts, gsizes)):
            if j < st0 + sz:
                return g, j - st0
        raise AssertionError(j)

    nc = bacc.Bacc("TRN2", target_bir_lowering=False, debug=False,
                   num_devices=N_CORES)
    qt_d = nc.dram_tensor("qt", [n_heads, 2 * D, s], F16, kind="ExternalInput")
    kt_d = nc.dram_tensor("kt", [n_heads, 2 * D, s], F16, kind="ExternalInput")
    v_d = nc.dram_tensor("v", [n_heads, s, D], F16, kind="ExternalInput")
    lam_d = nc.dram_tensor("lam", [n_heads, 1], F32, kind="ExternalInput")
    out_d = nc.dram_tensor("out", [n_heads, s, D], F32, kind="ExternalOutput")

    with tile.TileContext(nc) as tc:
        with (
            tc.tile_pool(name="consts", bufs=1) as consts,
            tc.tile_pool(name="qk", bufs=2) as qk_pool,
            tc.tile_pool(name="vx", bufs=2) as vx_pool,
            tc.tile_pool(name="lamp", bufs=2) as lam_pool,
            tc.tile_pool(name="u", bufs=u_bufs) as u_pool,
            tc.tile_pool(name="ud", bufs=12) as ud_pool,
            tc.tile_pool(name="u32", bufs=3) as u32_pool,
            tc.tile_pool(name="o", bufs=1) as o_pool,
            tc.tile_pool(name="tr", bufs=8) as tr_pool,
            tc.tile_pool(name="w", bufs=3) as w_pool,
            tc.tile_pool(name="stats", bufs=3) as stats_pool,
            tc.tile_pool(name="small", bufs=8) as small_pool,
            tc.tile_pool(name="ps_sc", bufs=1, space="PSUM") as ps_scores,
            tc.tile_pool(name="ps_av", bufs=2, space="PSUM") as ps_av,
        ):
            eps_ap = consts.tile([128, 1], F32)
            nc.vector.memset(eps_ap, EPS)
            # prefetch the exp table set while the first DMAs run
            warm = consts.tile([128, 1], F32)
            nc.scalar.activation(warm, eps_ap, AF.Exp)

            # Two static sets of O^T staging tiles (heads alternate) so the
            # xbar-transpose pad rows 67..79 are zeroed exactly once.
            o_static = []
            for par in range(2):
                o1s = o_pool.tile([OROWS, s], F16, tag=f"o1_{par}")
                o2s = o_pool.tile([OROWS, s], F16, tag=f"o2_{par}")
                nc.gpsimd.memset(o1s[64:OROWS, :], 0.0)
                nc.gpsimd.memset(o2s[64:OROWS, :], 0.0)
                o_static.append((o1s, o2s))

            head_state = {}
            pending_ops = []

            def load_head(h):
                qt_sb = qk_pool.tile([2 * D, s], F16, tag="qt")
                kt_sb = qk_pool.tile([2 * D, s], F16, tag="kt")
                if h == 0:
                    # minimal first slices so group 0 can start ~2us in
                    nc.sync.dma_start(out=kt_sb[:, 0:256], in_=kt_d[h][:, 0:256])
                    nc.sync.dma_start(out=qt_sb[:, 0:QC], in_=qt_d[h][:, 0:QC])
                    nc.sync.dma_start(out=kt_sb[:, 256:s], in_=kt_d[h][:, 256:s])
                    nc.sync.dma_start(out=qt_sb[:, QC:s], in_=qt_d[h][:, QC:s])
                else:
                    nc.sync.dma_start(out=qt_sb, in_=qt_d[h])
                    nc.sync.dma_start(out=kt_sb, in_=kt_d[h])
                # AV weights: [0, V0..61, 0, V62, V63, ones] per k tile
                vx = vx_pool.tile([128, nkt, MAV], F16, tag="vx")
                vsrc = v_d[h].rearrange("(t p) d -> p t d", p=128)
                nc.sync.dma_start(out=vx[:, :, 1:63], in_=vsrc[:, :, 0:62])
                nc.sync.dma_start(out=vx[:, :, 64:66], in_=vsrc[:, :, 62:64])
                nc.vector.memset(vx[:, :, 0:1], 0.0)
                nc.vector.memset(vx[:, :, 63:64], 0.0)
                nc.vector.memset(vx[:, :, RROW : RROW + 1], 1.0)
                lamneg = lam_pool.tile([128, 1], F32, tag="lam")
                nc.sync.dma_start(out=lamneg, in_=lam_d[h].to_broadcast((128, 1)))
                nc.vector.tensor_scalar_mul(lamneg, lamneg, -1.0)
                o1, o2 = o_static[h % 2]
                w_head = w_pool.tile([128, nqt, D], F32, tag="w")
                head_state[h] = dict(qt=qt_sb, kt=kt_sb, vx=vx, lamneg=lamneg,
                                     o1=o1, o2=o2, w=w_head, mv=None)

            def score_group(h, qc, g):
                """One group of score units -> one PSUM tile -> exp -> U.
                Unit j = (kt = j//2, half = j%2). ScalarE ACTIVATE(Exp)
                drains slots 0..2 of 4-bank tiles and both slots of 2-bank
                tiles; slot 3 of each 4-bank tile goes to the DVE as a
                fp32 copy to SBUF + int16 Schraudolph tensor_scalar."""
                st = head_state[h]
                qt_sb, kt_sb = st["qt"], st["kt"]
                j0, n = gstarts[g], gsizes[g]
                ps = ps_scores.tile(
                    [128, n * QC], F32, tag=("ps4" if n == 4 else "ps2"))
                for i in range(n):
                    j = j0 + i
                    kt = j // 2
                    rb = (j % 4) * D2
                    dsl = slice(rb, rb + D2)
                    nc.tensor.matmul(
                        ps[:, i * QC : (i + 1) * QC],
                        kt_sb[dsl, kt * KTILE : (kt + 1) * KTILE],
                        qt_sb[dsl, qc * QC : (qc + 1) * QC],
                        start=True, stop=True,
                        tile_position=(rb, 0),
                    )
                na = n - 1 if (DVE_SPLIT and n == 4) else n
                u = u_pool.tile([128, 3 * QC], F16, tag="u")
                nc.scalar.activation(
                    u[:, 0 : na * QC], ps[:, 0 : na * QC], AF.Exp,
                    scale=SCALE)
                udve = None
                if na < n:
                    t32 = u32_pool.tile([128, QC], F32, tag="u32")
                    nc.vector.tensor_copy(t32, ps[:, 3 * QC : 4 * QC])
                    udve = ud_pool.tile([128, QC], I16, tag="ud")
                    nc.vector.tensor_scalar(
                        out=udve, in0=t32,
                        scalar1=EXP_A, scalar2=EXP_B,
                        op0=ALU.mult, op1=ALU.add)
                st[("us", qc)].append((u, udve))

            def av_chunk(h, qc, mlist):
                """AV matmuls m in mlist; m = half*nkt + kt."""
                st = head_state[h]
                vx = st["vx"]
                us = st[("us", qc)]
                for m in mlist:
                    half, kt = m // nkt, m % nkt
                    if kt == 0:
                        pav_new = ps_av.tile([MAV, QC], F32, tag="pav")
                        st[("pav", qc, half)] = pav_new
                    pav = st[("pav", qc, half)]
                    g, slot = unit_slot(kt * 2 + half)
                    ua, udve = us[g]
                    if udve is not None and slot == 3:
                        u_ap = udve[:, :].bitcast(F16)
                    else:
                        u_ap = ua[:, slot * QC : (slot + 1) * QC]
                    nc.tensor.matmul(
                        pav, vx[:, kt, :], u_ap,
                        start=(kt == 0), stop=(kt == nkt - 1),
                    )
                    if kt == nkt - 1:
                        o_sb = st["o2"] if half else st["o1"]
                        nc.vector.tensor_copy(
                            o_sb[0:MAV, qc * QC : (qc + 1) * QC], pav)
                        del st[("pav", qc, half)]

            def norm_chunk(h, qc):
                """DMA-xbar transpose of one q chunk; the DVE combine+stats
                ops go through the pump so they interleave with the next
                step's exp work instead of blocking the ps2 drains.
                tr layout r-index: 1..62 = V0..61, 64,65 = V62,63, 66 = r."""
                st = head_state[h]
                o1, o2, lamneg = st["o1"], st["o2"], st["lamneg"]
                if st["mv"] is None:
                    mv_new = stats_pool.tile([128, nqt, 2], F32, tag="mv")
                    st["mv"] = mv_new
                mv = st["mv"]
                csl = slice(qc * QC, (qc + 1) * QC)
                tr1 = tr_pool.tile([128, tpq, OROWS], F16, tag="tr1")
                nc.sync.dma_start_transpose(tr1, o1[:, csl])
                tr2 = tr_pool.tile([128, tpq, OROWS], F16, tag="tr2")
                nc.sync.dma_start_transpose(tr2, o2[:, csl])
                r1v = small_pool.tile([128, tpq], F32, tag="r1v")
                r2v = small_pool.tile([128, tpq], F32, tag="r2v")
                w2 = small_pool.tile([128, tpq, D], F32, tag="w2")
                s6 = small_pool.tile([128, tpq, 6], F32, tag="s6")
                w = st["w"][:, qc * tpq : (qc + 1) * tpq, :]
                ops = [
                    lambda: nc.vector.reciprocal(r1v, tr1[:, :, RROW : RROW + 1]),
                    lambda: nc.vector.reciprocal(r2v, tr2[:, :, RROW : RROW + 1]),
                    lambda: nc.vector.tensor_scalar_mul(r2v, r2v, lamneg),
                ]
                for lo, hi, tlo in ((0, 62, 1), (62, 64, 64)):
                    nw = hi - lo
                    r1b = r1v[:][:, :, None].broadcast_to((128, tpq, nw))
                    r2b = r2v[:][:, :, None].broadcast_to((128, tpq, nw))
                    ops += [
                        lambda lo=lo, hi=hi, tlo=tlo, nw=nw, r1b=r1b:
                            nc.vector.tensor_mul(
                                w[:, :, lo:hi], tr1[:, :, tlo : tlo + nw], r1b),
                        lambda lo=lo, hi=hi, tlo=tlo, nw=nw, r2b=r2b:
                            nc.vector.tensor_mul(
                                w2[:, :, lo:hi], tr2[:, :, tlo : tlo + nw], r2b),
                        lambda lo=lo, hi=hi:
                            nc.vector.tensor_add(
                                w[:, :, lo:hi], w[:, :, lo:hi], w2[:, :, lo:hi]),
                    ]
                for t in range(tpq):
                    ops += [
                        lambda t=t: nc.vector.bn_stats(
                            out=s6[:, t, :], in_=w[:, t, :]),
                        lambda t=t: nc.vector.bn_aggr(
                            out=mv[:, qc * tpq + t, :], in_=s6[:, t, :]),
                    ]
                return ops

            def finish_head_a(h):
                """rstd = (1-lam0)*rsqrt(var+eps): Schraudolph ln seed ->
                fp16 Schraudolph exp -> 2 Newton steps (last folds the
                (1-lam0) factor). Ops dribbled between score groups."""
                st = head_state[h]
                mv = st["mv"]
                x = stats_pool.tile([128, nqt], F32, tag="x")
                sd = stats_pool.tile([128, nqt], F32, tag="sd")
                y16 = stats_pool.tile([128, nqt], F16, tag="y16")
                y = stats_pool.tile([128, nqt], F32, tag="y")
                t = stats_pool.tile([128, nqt], F32, tag="t")
                st["y"] = y
                c = 1.0 - LAMBDA_INIT
                ops = [
                    lambda: nc.vector.tensor_scalar_add(x, mv[:, :, 1], EPS),
                    lambda: nc.vector.tensor_scalar(
                        out=sd, in0=x[:].bitcast(I32),
                        scalar1=RS_K1, scalar2=RS_K2,
                        op0=ALU.mult, op1=ALU.add),
                    lambda: nc.vector.tensor_scalar(
                        out=y16[:].bitcast(I16), in0=sd,
                        scalar1=RS_A, scalar2=RS_B,
                        op0=ALU.mult, op1=ALU.add),
                    # Newton 1: y = y16*(1.5 - 0.5*x*y16^2)
                    lambda: nc.vector.tensor_mul(t, y16, y16),
                    lambda: nc.vector.tensor_mul(t, t, x),
                    lambda: nc.vector.tensor_scalar(
                        out=t, in0=t, scalar1=-0.5, scalar2=1.5,
                        op0=ALU.mult, op1=ALU.add),
                    lambda: nc.vector.tensor_mul(y, y16, t),
                    # Newton 2 with (1-lam0) folded in
                    lambda: nc.vector.tensor_mul(t, y, y),
                    lambda: nc.vector.tensor_mul(t, t, x),
                    lambda: nc.vector.tensor_scalar(
                        out=t, in0=t, scalar1=-0.5 * c, scalar2=1.5 * c,
                        op0=ALU.mult, op1=ALU.add),
                    lambda: nc.vector.tensor_mul(y, y, t),
                ]
                pending_ops.extend(ops)

            def finish_head_b(h):
                st = head_state[h]
                mv, y = st["mv"], st["y"]
                m2 = stats_pool.tile([128, nqt], F32, tag="m2")
                w3 = st["w"][:, :, :]
                yb = y[:][:, :, None].broadcast_to((128, nqt, D))
                m2b = m2[:][:, :, None].broadcast_to((128, nqt, D))
                wd = out_d[h].rearrange("(t p) d -> p t d", p=128)
                pending_ops.extend([
                    lambda: nc.vector.tensor_mul(m2, mv[:, :, 0], y),
                    lambda: nc.vector.tensor_mul(w3, w3, yb),
                    lambda: nc.vector.tensor_sub(w3, w3, m2b),
                    lambda: nc.gpsimd.dma_start(out=wd, in_=w3),
                    lambda: head_state.pop(h),
                ])

            def pump(n):
                for _ in range(min(n, len(pending_ops))):
                    pending_ops.pop(0)()

            # ---- emission: one flat (head, chunk) pipeline; scores of step
            # s+1 interleave with AV of step s across head boundaries ----
            load_head(0)
            deferred_norm = []
            nsteps = n_heads * nq
            for step in range(nsteps + 1):
                if step < nsteps:
                    h, qc = divmod(step, nq)
                    head_state[h][("us", qc)] = []
                else:
                    h = qc = None
                ph, pqc = divmod(step - 1, nq)
                for g in range(ngrp + 1):
                    if step < nsteps and g < ngrp:
                        score_group(h, qc, g)
                    if step > 0 and g > 0:
                        av_chunk(ph, pqc,
                                 range(gstarts[g - 1],
                                       gstarts[g - 1] + gsizes[g - 1]))
                    if g >= 3 or step == nsteps:
                        pump(4)
                if step > 0:
                    head_state[ph].pop(("us", pqc))
                    # norm DVE ops deferred one extra step so the transposes
                    # are long done before the pumped ops reach the queue
                    pending_ops.extend(deferred_norm)
                    deferred_norm = norm_chunk(ph, pqc)
                    if pqc == min(1, nq - 1) and ph > 0:
                        finish_head_a(ph - 1)
                    if pqc == min(2, nq - 1) and ph > 0:
                        finish_head_b(ph - 1)
                if step < nsteps and qc == nq - 2 and h + 1 < n_heads:
                    load_head(h + 1)  # prefetch next head's tensors
            pending_ops.extend(deferred_norm)
            finish_head_a(n_heads - 1)
            finish_head_b(n_heads - 1)
            pump(len(pending_ops))

    nc.compile()
    return nc


_PROGRAM_CACHE = {}


def _get_program():
    key = (HEADS_PER_CORE, S)
    if key not in _PROGRAM_CACHE:
        _PROGRAM_CACHE[key] = build_program()
    return _PROGRAM_CACHE[key]


def shard_inputs(query, key, value, lambda_params):
    """Full [B,H,S,D] inputs -> per-core input maps (host-side prep)."""
    q = np.asarray(query, dtype=np.float32).reshape(B * H, S, D)
    k = np.asarray(key, dtype=np.float32).reshape(B * H, S, D)
    v = np.asarray(value, dtype=np.float32).reshape(B * H, S, D)
    lam = np.asarray(lambda_params, dtype=np.float32)
    lam_full = np.tile(lam, B)  # pair i = (b=i//H, h=i%H) -> lambda[i%H]
    in_maps = []
    for c in range(N_CORES):
        sl = slice(c * HEADS_PER_CORE, (c + 1) * HEADS_PER_CORE)
        qt = q[sl].transpose(0, 2, 1).astype(np.float16)
        kt = k[sl].transpose(0, 2, 1).astype(np.float16)
        in_maps.append({
            "qt": np.ascontiguousarray(np.concatenate([qt, qt], axis=1)),
            "kt": np.ascontiguousarray(np.concatenate([kt, kt], axis=1)),
            "v": np.ascontiguousarray(v[sl]).astype(np.float16),
            "lam": np.ascontiguousarray(lam_full[sl].reshape(-1, 1)),
        })
    return in_maps


def kernel(query, key, value, lambda_params, trace=False):
    nc = _get_program()
    in_maps = shard_inputs(query, key, value, lambda_params)
    res = run_bass_kernel_spmd(nc, in_maps, core_ids=list(range(N_CORES)),
                               trace=trace)
    out = np.concatenate([r["out"] for r in res.results], axis=0)
    out = out.reshape(B, H, S, D).astype(np.float32)
    if trace:
        kernel.last_exec_time_ns = res.exec_time_ns
        kernel.last_results = res
    return out


# revision 26
# speedup vs baseline: 1.1736x; 1.1736x over previous
"""Differential attention kernel for Trainium2 (8 NeuronCores, SPMD).

Math per (batch, head):
    q1,q2 / k1,k2 = halves of head_dim (D=64 -> d2=32)
    a_i = softmax(q_i @ k_i^T / sqrt(d2))        (i = 1,2)
    out = (a1 - lam*a2) @ V, then per-(q) groupnorm over D, scaled by (1-0.8).

Design (per core: 4 of the 32 (b,h) pairs). ~34M exp evals/core dominate;
they are split across TWO engines: ScalarE ACTIVATE(Exp) drains the 4-bank
PSUM score tiles, and the DVE drains four of the six 2-bank tiles with a
one-instruction Schraudolph fast-exp: u16 = int16(s*A + B) bit-viewed as
fp16 (A = 1024*log2e*scale, B = 15*1024-45; |rel err| <= 3%, applied to
25% of scores; final output err ~1.4e-2 < 2e-2 tolerance, sim-verified).
  - Q/K/V cast to fp16 on host; Q^T/K^T shipped twice ([2D, s]) so 4 score
    matmuls (K=32 contraction) run concurrently in the four 32-row PE row
    groups via tile_position.
  - Scores transposed: S^T[k, q] units [128, 512] packed in alternating
    4-bank (ScalarE) / 2-bank PSUM tiles; 2 remaining banks double-buffer
    the AV accumulators. exp needs no max-subtraction: scores ~ N(0,1).
  - U^T fp16 in SBUF; AV lhsT = [0 | V0..61 | 0 | V62 V63 | ones] (M=67).
    The dummy columns at positions 0 and 63 absorb a hardware hazard
    observed under ACT+DVE+PE concurrent PSUM access: AV-matmul outputs at
    PSUM partitions 0 and 63 are intermittently corrupted, so no real data
    lives there. The ones column accumulates softmax row-sums for free.
  - O^T[67, q] per 512-q chunk, fp16 to SBUF (padded to 80 rows), DMA xbar
    transpose to natural layout.
  - Norm path batched per chunk: 2 strided reciprocals + 1 scale + 6
    tensor_tensor (W = O1*r1inv - lam*O2*r2inv via step-0 broadcast APs,
    in a 62-wide and a 2-wide piece around the dummy row) + 4 bn_stats +
    4 bn_aggr.
  - rstd = (1-lam0)*rsqrt(var+eps) via Schraudolph ln seed + fp16
    Schraudolph exp + 2 Newton steps, dribbled between score groups.
"""

import math
import numpy as np

import concourse.bass as bass
import concourse.tile as tile
from concourse import bacc, mybir
from concourse.bass_utils import run_bass_kernel_spmd

F32 = mybir.dt.float32
F16 = mybir.dt.float16
I16 = mybir.dt.int16
I32 = mybir.dt.int32
AF = mybir.ActivationFunctionType
ALU = mybir.AluOpType

B, H, S, D = 2, 16, 2048, 64
D2 = D // 2
N_CORES = 8
HEADS_PER_CORE = (B * H) // N_CORES  # 4
LAMBDA_INIT = 0.8
EPS = 1e-5
SCALE = 1.0 / math.sqrt(D2)
LOG2E = 1.4426950408889634
LN2 = 0.6931471805599453

QC = 512           # q chunk (one PSUM bank of fp32)
KTILE = 128        # k tile (partition dim)
MAV = 67           # AV out rows: [dummy, V0..61, dummy, V62, V63, ones]
RROW = 66          # row-sum (ones) position in O^T
OROWS = 80         # O^T rows padded to xbar 16-row granularity

# Schraudolph fast-exp constants (int16 bits of fp16): round(t*EXP_A + EXP_B)
EXP_C = 45.0
EXP_A = 1024.0 * LOG2E * SCALE
EXP_B = 15.0 * 1024.0 - EXP_C
# rsqrt seed: ln(x) ~ (bits(x)*2^-23 - 127 + 0.043)*ln2;  seed = exp(-ln(x)/2)
RS_K1 = -0.5 * LN2 / (1 << 23)
RS_K2 = 0.5 * LN2 * (127.0 - 0.043)
RS_A = 1024.0 * LOG2E
RS_B = 15.0 * 1024.0 - EXP_C

DVE_SPLIT = False  # slot 3 of each 4-bank tile exp'd on the DVE (f=0.1875)


def _group_sizes(nunits):
    """Score units per PSUM tile: [4,2]*4 + [4,4]. The 4-bank (ps4) and
    2-bank (ps2) tiles alternate so ScalarE ping-pongs between two buffers
    (the PE refills one while ScalarE drains the other). Slot 3 of each
    4-bank tile is drained by the DVE instead (Schraudolph)."""
    assert nunits == 32
    return [4, 2, 4, 2, 4, 2, 4, 2, 4, 4]


def build_program(n_heads=HEADS_PER_CORE, s=S, u_bufs=26):
    nq = s // QC          # q chunks per head
    nkt = s // KTILE      # k tiles per head
    nqt = s // 128        # q tiles (norm phase)
    tpq = QC // 128       # q tiles per chunk
    nunits = 2 * nkt      # score units per q chunk (2 halves x k tiles)
    gsizes = _group_sizes(nunits)
    gstarts = [sum(gsizes[:i]) for i in range(len(gsizes))]
    ngrp = len(gsizes)

    def unit_slot(j):
        """U-tile index and within-tile slot for score unit j."""
        for g, (st0, sz) in enumerate(zip(gstarts, gsizes)):
            if j < st0 + sz:
                return g, j - st0
        raise AssertionError(j)

    nc = bacc.Bacc("TRN2", target_bir_lowering=False, debug=False,
                   num_devices=N_CORES)
    qt_d = nc.dram_tensor("qt", [n_heads, 2 * D, s], F16, kind="ExternalInput")
    kt_d = nc.dram_tensor("kt", [n_heads, 2 * D, s], F16, kind="ExternalInput")
    v_d = nc.dram_tensor("v", [n_heads, s, D], F16, kind="ExternalInput")
    lam_d = nc.dram_tensor("lam", [n_heads, 1], F32, kind="ExternalInput")
    out_d = nc.dram_tensor("out", [n_heads, s, D], F32, kind="ExternalOutput")

    with tile.TileContext(nc) as tc:
        with (
            tc.tile_pool(name="consts", bufs=1) as consts,
            tc.tile_pool(name="qk", bufs=2) as qk_pool,
            tc.tile_pool(name="vx", bufs=2) as vx_pool,
            tc.tile_pool(name="lamp", bufs=2) as lam_pool,
            tc.tile_pool(name="u", bufs=u_bufs) as u_pool,
            tc.tile_pool(name="ud", bufs=12) as ud_pool,
            tc.tile_pool(name="u32", bufs=3) as u32_pool,
            tc.tile_pool(name="o", bufs=1) as o_pool,
            tc.tile_pool(name="tr", bufs=8) as tr_pool,
            tc.tile_pool(name="w", bufs=3) as w_pool,
            tc.tile_pool(name="stats", bufs=3) as stats_pool,
            tc.tile_pool(name="small", bufs=8) as small_pool,
            tc.tile_pool(name="ps_sc", bufs=1, space="PSUM") as ps_scores,
            tc.tile_pool(name="ps_av", bufs=2, space="PSUM") as ps_av,
        ):
            eps_ap = consts.tile([128, 1], F32)
            nc.vector.memset(eps_ap, EPS)
            # prefetch the exp table set while the first DMAs run
            warm = consts.tile([128, 1], F32)
            nc.scalar.activation(warm, eps_ap, AF.Exp)

            # Two static sets of O^T staging tiles (heads alternate) so the
            # xbar-transpose pad rows 67..79 are zeroed exactly once.
            o_static = []
            for par in range(2):
                o1s = o_pool.tile([OROWS, s], F16, tag=f"o1_{par}")
                o2s = o_pool.tile([OROWS, s], F16, tag=f"o2_{par}")
                nc.gpsimd.memset(o1s[64:OROWS, :], 0.0)
                nc.gpsimd.memset(o2s[64:OROWS, :], 0.0)
                o_static.append((o1s, o2s))

            head_state = {}
            pending_ops = []

            def load_head(h):
                qt_sb = qk_pool.tile([2 * D, s], F16, tag="qt")
                kt_sb = qk_pool.tile([2 * D, s], F16, tag="kt")
                if h == 0:
                    # minimal first slices so group 0 can start ~2us in
                    nc.sync.dma_start(out=kt_sb[:, 0:256], in_=kt_d[h][:, 0:256])
                    nc.sync.dma_start(out=qt_sb[:, 0:QC], in_=qt_d[h][:, 0:QC])
                    nc.sync.dma_start(out=kt_sb[:, 256:s], in_=kt_d[h][:, 256:s])
                    nc.sync.dma_start(out=qt_sb[:, QC:s], in_=qt_d[h][:, QC:s])
                else:
                    nc.sync.dma_start(out=qt_sb, in_=qt_d[h])
                    nc.sync.dma_start(out=kt_sb, in_=kt_d[h])
                # AV weights: [0, V0..61, 0, V62, V63, ones] per k tile
                vx = vx_pool.tile([128, nkt, MAV], F16, tag="vx")
                vsrc = v_d[h].rearrange("(t p) d -> p t d", p=128)
                nc.sync.dma_start(out=vx[:, :, 1:63], in_=vsrc[:, :, 0:62])
                nc.sync.dma_start(out=vx[:, :, 64:66], in_=vsrc[:, :, 62:64])
                nc.vector.memset(vx[:, :, 0:1], 0.0)
                nc.vector.memset(vx[:, :, 63:64], 0.0)
                nc.vector.memset(vx[:, :, RROW : RROW + 1], 1.0)
                lamneg = lam_pool.tile([128, 1], F32, tag="lam")
                nc.sync.dma_start(out=lamneg, in_=lam_d[h].to_broadcast((128, 1)))
                nc.vector.tensor_scalar_mul(lamneg, lamneg, -1.0)
                o1, o2 = o_static[h % 2]
                w_head = w_pool.tile([128, nqt, D], F32, tag="w")
                head_state[h] = dict(qt=qt_sb, kt=kt_sb, vx=vx, lamneg=lamneg,
                                     o1=o1, o2=o2, w=w_head, mv=None)

            def score_group(h, qc, g):
                """One group of score units -> one PSUM tile -> exp -> U.
                Unit j = (kt = j//2, half = j%2). ScalarE ACTIVATE(Exp)
                drains slots 0..2 of 4-bank tiles and both slots of 2-bank
                tiles; slot 3 of each 4-bank tile goes to the DVE as a
                fp32 copy to SBUF + int16 Schraudolph tensor_scalar."""
                st = head_state[h]
                qt_sb, kt_sb = st["qt"], st["kt"]
                j0, n = gstarts[g], gsizes[g]
                ps = ps_scores.tile(
                    [128, n * QC], F32, tag=("ps4" if n == 4 else "ps2"))
                for i in range(n):
                    j = j0 + i
                    kt = j // 2
                    rb = (j % 4) * D2
                    dsl = slice(rb, rb + D2)
                    nc.tensor.matmul(
                        ps[:, i * QC : (i + 1) * QC],
                        kt_sb[dsl, kt * KTILE : (kt + 1) * KTILE],
                        qt_sb[dsl, qc * QC : (qc + 1) * QC],
                        start=True, stop=True,
                        tile_position=(rb, 0),
                    )
                na = n - 1 if (DVE_SPLIT and n == 4) else n
                u = u_pool.tile([128, (3 if DVE_SPLIT else 4) * QC], F16, tag="u")
                nc.scalar.activation(
                    u[:, 0 : na * QC], ps[:, 0 : na * QC], AF.Exp,
                    scale=SCALE)
                udve = None
                if na < n:
                    t32 = u32_pool.tile([128, QC], F32, tag="u32")
                    nc.vector.tensor_copy(t32, ps[:, 3 * QC : 4 * QC])
                    udve = ud_pool.tile([128, QC], I16, tag="ud")
                    nc.vector.tensor_scalar(
                        out=udve, in0=t32,
                        scalar1=EXP_A, scalar2=EXP_B,
                        op0=ALU.mult, op1=ALU.add)
                st[("us", qc)].append((u, udve))

            def av_chunk(h, qc, mlist):
                """AV matmuls m in mlist; m = half*nkt + kt."""
                st = head_state[h]
                vx = st["vx"]
                us = st[("us", qc)]
                for m in mlist:
                    half, kt = m // nkt, m % nkt
                    if kt == 0:
                        pav_new = ps_av.tile([MAV, QC], F32, tag="pav")
                        st[("pav", qc, half)] = pav_new
                    pav = st[("pav", qc, half)]
                    g, slot = unit_slot(kt * 2 + half)
                    ua, udve = us[g]
                    if udve is not None and slot == 3:
                        u_ap = udve[:, :].bitcast(F16)
                    else:
                        u_ap = ua[:, slot * QC : (slot + 1) * QC]
                    nc.tensor.matmul(
                        pav, vx[:, kt, :], u_ap,
                        start=(kt == 0), stop=(kt == nkt - 1),
                    )
                    if kt == nkt - 1:
                        o_sb = st["o2"] if half else st["o1"]
                        nc.vector.tensor_copy(
                            o_sb[0:MAV, qc * QC : (qc + 1) * QC], pav)
                        del st[("pav", qc, half)]

            def norm_chunk(h, qc):
                """DMA-xbar transpose of one q chunk; the DVE combine+stats
                ops go through the pump so they interleave with the next
                step's exp work instead of blocking the ps2 drains.
                tr layout r-index: 1..62 = V0..61, 64,65 = V62,63, 66 = r."""
                st = head_state[h]
                o1, o2, lamneg = st["o1"], st["o2"], st["lamneg"]
                if st["mv"] is None:
                    mv_new = stats_pool.tile([128, nqt, 2], F32, tag="mv")
                    st["mv"] = mv_new
                mv = st["mv"]
                csl = slice(qc * QC, (qc + 1) * QC)
                tr1 = tr_pool.tile([128, tpq, OROWS], F16, tag="tr1")
                nc.sync.dma_start_transpose(tr1, o1[:, csl])
                tr2 = tr_pool.tile([128, tpq, OROWS], F16, tag="tr2")
                nc.sync.dma_start_transpose(tr2, o2[:, csl])
                r1v = small_pool.tile([128, tpq], F32, tag="r1v")
                r2v = small_pool.tile([128, tpq], F32, tag="r2v")
                w2 = small_pool.tile([128, tpq, D], F32, tag="w2")
                s6 = small_pool.tile([128, tpq, 6], F32, tag="s6")
                w = st["w"][:, qc * tpq : (qc + 1) * tpq, :]
                ops = [
                    lambda: nc.vector.reciprocal(r1v, tr1[:, :, RROW : RROW + 1]),
                    lambda: nc.vector.reciprocal(r2v, tr2[:, :, RROW : RROW + 1]),
                    lambda: nc.vector.tensor_scalar_mul(r2v, r2v, lamneg),
                ]
                for lo, hi, tlo in ((0, 62, 1), (62, 64, 64)):
                    nw = hi - lo
                    r1b = r1v[:][:, :, None].broadcast_to((128, tpq, nw))
                    r2b = r2v[:][:, :, None].broadcast_to((128, tpq, nw))
                    ops += [
                        lambda lo=lo, hi=hi, tlo=tlo, nw=nw, r1b=r1b:
                            nc.vector.tensor_mul(
                                w[:, :, lo:hi], tr1[:, :, tlo : tlo + nw], r1b),
                        lambda lo=lo, hi=hi, tlo=tlo, nw=nw, r2b=r2b:
                            nc.vector.tensor_mul(
                                w2[:, :, lo:hi], tr2[:, :, tlo : tlo + nw], r2b),
                        lambda lo=lo, hi=hi:
                            nc.vector.tensor_add(
                                w[:, :, lo:hi], w[:, :, lo:hi], w2[:, :, lo:hi]),
                    ]
                for t in range(tpq):
                    ops += [
                        lambda t=t: nc.vector.bn_stats(
                            out=s6[:, t, :], in_=w[:, t, :]),
                        lambda t=t: nc.vector.bn_aggr(
                            out=mv[:, qc * tpq + t, :], in_=s6[:, t, :]),
                    ]
                return ops

            def finish_head_a(h):
                """rstd = (1-lam0)*rsqrt(var+eps): Schraudolph ln seed ->
                fp16 Schraudolph exp -> 2 Newton steps (last folds the
                (1-lam0) factor). Ops dribbled between score groups."""
                st = head_state[h]
                mv = st["mv"]
                x = stats_pool.tile([128, nqt], F32, tag="x")
                sd = stats_pool.tile([128, nqt], F32, tag="sd")
                y16 = stats_pool.tile([128, nqt], F16, tag="y16")
                y = stats_pool.tile([128, nqt], F32, tag="y")
                t = stats_pool.tile([128, nqt], F32, tag="t")
                st["y"] = y
                c = 1.0 - LAMBDA_INIT
                ops = [
                    lambda: nc.vector.tensor_scalar_add(x, mv[:, :, 1], EPS),
                    lambda: nc.vector.tensor_scalar(
                        out=sd, in0=x[:].bitcast(I32),
                        scalar1=RS_K1, scalar2=RS_K2,
                        op0=ALU.mult, op1=ALU.add),
                    lambda: nc.vector.tensor_scalar(
                        out=y16[:].bitcast(I16), in0=sd,
                        scalar1=RS_A, scalar2=RS_B,
                        op0=ALU.mult, op1=ALU.add),
                    # Newton 1: y = y16*(1.5 - 0.5*x*y16^2)
                    lambda: nc.vector.tensor_mul(t, y16, y16),
                    lambda: nc.vector.tensor_mul(t, t, x),
                    lambda: nc.vector.tensor_scalar(
                        out=t, in0=t, scalar1=-0.5, scalar2=1.5,
                        op0=ALU.mult, op1=ALU.add),
                    lambda: nc.vector.tensor_mul(y, y16, t),
                    # Newton 2 with (1-lam0) folded in
                    lambda: nc.vector.tensor_mul(t, y, y),
                    lambda: nc.vector.tensor_mul(t, t, x),
                    lambda: nc.vector.tensor_scalar(
                        out=t, in0=t, scalar1=-0.5 * c, scalar2=1.5 * c,
                        op0=ALU.mult, op1=ALU.add),
                    lambda: nc.vector.tensor_mul(y, y, t),
                ]
                pending_ops.extend(ops)

            def finish_head_b(h):
                st = head_state[h]
                mv, y = st["mv"], st["y"]
                m2 = stats_pool.tile([128, nqt], F32, tag="m2")
                w3 = st["w"][:, :, :]
                yb = y[:][:, :, None].broadcast_to((128, nqt, D))
                m2b = m2[:][:, :, None].broadcast_to((128, nqt, D))
                wd = out_d[h].rearrange("(t p) d -> p t d", p=128)
                pending_ops.extend([
                    lambda: nc.vector.tensor_mul(m2, mv[:, :, 0], y),
                    lambda: nc.vector.tensor_mul(w3, w3, yb),
                    lambda: nc.vector.tensor_sub(w3, w3, m2b),
                    lambda: nc.gpsimd.dma_start(out=wd, in_=w3),
                    lambda: head_state.pop(h),
                ])

            def pump(n):
                for _ in range(min(n, len(pending_ops))):
                    pending_ops.pop(0)()

            # ---- emission: one flat (head, chunk) pipeline; scores of step
            # s+1 interleave with AV of step s across head boundaries ----
            load_head(0)
            deferred_norm = []
            nsteps = n_heads * nq
            for step in range(nsteps + 1):
                if step < nsteps:
                    h, qc = divmod(step, nq)
                    head_state[h][("us", qc)] = []
                else:
                    h = qc = None
                ph, pqc = divmod(step - 1, nq)
                if step == nsteps:
                    for gl in range(ngrp - 5, ngrp):
                        av_chunk(ph, pqc,
                                 range(gstarts[gl], gstarts[gl] + gsizes[gl]))
                last = step == nsteps - 1
                for g in range(ngrp + 1):
                    if step < nsteps and g < ngrp:
                        score_group(h, qc, g)
                    if step > 0 and g > 0 and step < nsteps:
                        av_chunk(ph, pqc,
                                 range(gstarts[g - 1],
                                       gstarts[g - 1] + gsizes[g - 1]))
                    if last and g >= 5:
                        # final chunk: start its AV early (5 groups behind
                        # its scores) so the tail after the last exp shrinks
                        gl = g - 5
                        av_chunk(h, qc,
                                 range(gstarts[gl], gstarts[gl] + gsizes[gl]))
                    if g >= 3 or step == nsteps:
                        pump(4)
                if step > 0:
                    head_state[ph].pop(("us", pqc))
                    # norm DVE ops deferred one extra step so the transposes
                    # are long done before the pumped ops reach the queue
                    pending_ops.extend(deferred_norm)
                    deferred_norm = norm_chunk(ph, pqc)
                    if pqc == min(1, nq - 1) and ph > 0:
                        finish_head_a(ph - 1)
                    if pqc == min(2, nq - 1) and ph > 0:
                        finish_head_b(ph - 1)
                if step < nsteps and qc == nq - 2 and h + 1 < n_heads:
                    load_head(h + 1)  # prefetch next head's tensors
            pending_ops.extend(deferred_norm)
            finish_head_a(n_heads - 1)
            finish_head_b(n_heads - 1)
            pump(len(pending_ops))

    nc.compile()
    return nc


_PROGRAM_CACHE = {}


def _get_program():
    key = (HEADS_PER_CORE, S)
    if key not in _PROGRAM_CACHE:
        _PROGRAM_CACHE[key] = build_program()
    return _PROGRAM_CACHE[key]


def shard_inputs(query, key, value, lambda_params):
    """Full [B,H,S,D] inputs -> per-core input maps (host-side prep)."""
    q = np.asarray(query, dtype=np.float32).reshape(B * H, S, D)
    k = np.asarray(key, dtype=np.float32).reshape(B * H, S, D)
    v = np.asarray(value, dtype=np.float32).reshape(B * H, S, D)
    lam = np.asarray(lambda_params, dtype=np.float32)
    lam_full = np.tile(lam, B)  # pair i = (b=i//H, h=i%H) -> lambda[i%H]
    in_maps = []
    for c in range(N_CORES):
        sl = slice(c * HEADS_PER_CORE, (c + 1) * HEADS_PER_CORE)
        qt = q[sl].transpose(0, 2, 1).astype(np.float16)
        kt = k[sl].transpose(0, 2, 1).astype(np.float16)
        in_maps.append({
            "qt": np.ascontiguousarray(np.concatenate([qt, qt], axis=1)),
            "kt": np.ascontiguousarray(np.concatenate([kt, kt], axis=1)),
            "v": np.ascontiguousarray(v[sl]).astype(np.float16),
            "lam": np.ascontiguousarray(lam_full[sl].reshape(-1, 1)),
        })
    return in_maps


def kernel(query, key, value, lambda_params, trace=False):
    nc = _get_program()
    in_maps = shard_inputs(query, key, value, lambda_params)
    res = run_bass_kernel_spmd(nc, in_maps, core_ids=list(range(N_CORES)),
                               trace=trace)
    out = np.concatenate([r["out"] for r in res.results], axis=0)
    out = out.reshape(B, H, S, D).astype(np.float32)
    if trace:
        kernel.last_exec_time_ns = res.exec_time_ns
        kernel.last_results = res
    return out


# revision 27
# speedup vs baseline: 1.2405x; 1.0570x over previous
"""Differential attention kernel for Trainium2 (8 NeuronCores, SPMD).

Math per (batch, head):
    q1,q2 / k1,k2 = halves of head_dim (D=64 -> d2=32)
    a_i = softmax(q_i @ k_i^T / sqrt(d2))        (i = 1,2)
    out = (a1 - lam*a2) @ V, then per-(q) groupnorm over D, scaled by (1-0.8).

Design (per core: 4 of the 32 (b,h) pairs). ~34M exp evals/core dominate;
they are split across TWO engines: ScalarE ACTIVATE(Exp) drains the 4-bank
PSUM score tiles, and the DVE drains four of the six 2-bank tiles with a
one-instruction Schraudolph fast-exp: u16 = int16(s*A + B) bit-viewed as
fp16 (A = 1024*log2e*scale, B = 15*1024-45; |rel err| <= 3%, applied to
25% of scores; final output err ~1.4e-2 < 2e-2 tolerance, sim-verified).
  - Q/K/V cast to fp16 on host; Q^T/K^T shipped twice ([2D, s]) so 4 score
    matmuls (K=32 contraction) run concurrently in the four 32-row PE row
    groups via tile_position.
  - Scores transposed: S^T[k, q] units [128, 512] packed in alternating
    4-bank (ScalarE) / 2-bank PSUM tiles; 2 remaining banks double-buffer
    the AV accumulators. exp needs no max-subtraction: scores ~ N(0,1).
  - U^T fp16 in SBUF; AV lhsT = [0 | V0..61 | 0 | V62 V63 | ones] (M=67).
    The dummy columns at positions 0 and 63 absorb a hardware hazard
    observed under ACT+DVE+PE concurrent PSUM access: AV-matmul outputs at
    PSUM partitions 0 and 63 are intermittently corrupted, so no real data
    lives there. The ones column accumulates softmax row-sums for free.
  - O^T[67, q] per 512-q chunk, fp16 to SBUF (padded to 80 rows), DMA xbar
    transpose to natural layout.
  - Norm path batched per chunk: 2 strided reciprocals + 1 scale + 6
    tensor_tensor (W = O1*r1inv - lam*O2*r2inv via step-0 broadcast APs,
    in a 62-wide and a 2-wide piece around the dummy row) + 4 bn_stats +
    4 bn_aggr.
  - rstd = (1-lam0)*rsqrt(var+eps) via Schraudolph ln seed + fp16
    Schraudolph exp + 2 Newton steps, dribbled between score groups.
"""

import math
import numpy as np

import concourse.bass as bass
import concourse.tile as tile
from concourse import bacc, mybir
from concourse.bass_utils import run_bass_kernel_spmd

F32 = mybir.dt.float32
F16 = mybir.dt.float16
I16 = mybir.dt.int16
I32 = mybir.dt.int32
AF = mybir.ActivationFunctionType
ALU = mybir.AluOpType

B, H, S, D = 2, 16, 2048, 64
D2 = D // 2
N_CORES = 8
HEADS_PER_CORE = (B * H) // N_CORES  # 4
LAMBDA_INIT = 0.8
EPS = 1e-5
SCALE = 1.0 / math.sqrt(D2)
LOG2E = 1.4426950408889634
LN2 = 0.6931471805599453

QC = 512           # q chunk (one PSUM bank of fp32)
KTILE = 128        # k tile (partition dim)
MAV = 67           # AV out rows: [dummy, V0..61, dummy, V62, V63, ones]
RROW = 66          # row-sum (ones) position in O^T
OROWS = 80         # O^T rows padded to xbar 16-row granularity

# Schraudolph fast-exp constants (int16 bits of fp16): round(t*EXP_A + EXP_B)
EXP_C = 45.0
EXP_A = 1024.0 * LOG2E * SCALE
EXP_B = 15.0 * 1024.0 - EXP_C
# rsqrt seed: ln(x) ~ (bits(x)*2^-23 - 127 + 0.043)*ln2;  seed = exp(-ln(x)/2)
RS_K1 = -0.5 * LN2 / (1 << 23)
RS_K2 = 0.5 * LN2 * (127.0 - 0.043)
RS_A = 1024.0 * LOG2E
RS_B = 15.0 * 1024.0 - EXP_C

DVE_SPLIT = False  # slot 3 of each 4-bank tile exp'd on the DVE (f=0.1875)


def _group_sizes(nunits):
    """Score units per PSUM tile / exp instruction: alternate 2- and 4-bank
    tiles (6 PSUM banks; 2 accumulate AV double-buffered). ScalarE
    ping-pongs between the two tile tags, so the PE refills one while
    ScalarE drains the other."""
    sizes = []
    while sum(sizes) < nunits:
        want = 4 if len(sizes) % 2 else 2
        sizes.append(min(want, nunits - sum(sizes)))
    return sizes


def build_program(n_heads=HEADS_PER_CORE, s=S, u_bufs=26):
    nq = s // QC          # q chunks per head
    nkt = s // KTILE      # k tiles per head
    nqt = s // 128        # q tiles (norm phase)
    tpq = QC // 128       # q tiles per chunk
    nunits = 2 * nkt      # score units per q chunk (2 halves x k tiles)
    gsizes = _group_sizes(nunits)
    gstarts = [sum(gsizes[:i]) for i in range(len(gsizes))]
    ngrp = len(gsizes)

    def unit_slot(j):
        """U-tile index and within-tile slot for score unit j."""
        for g, (st0, sz) in enumerate(zip(gstarts, gsizes)):
            if j < st0 + sz:
                return g, j - st0
        raise AssertionError(j)

    nc = bacc.Bacc("TRN2", target_bir_lowering=False, debug=False,
                   num_devices=N_CORES)
    qt_d = nc.dram_tensor("qt", [n_heads, 2 * D, s], F16, kind="ExternalInput")
    kt_d = nc.dram_tensor("kt", [n_heads, 2 * D, s], F16, kind="ExternalInput")
    v_d = nc.dram_tensor("v", [n_heads, s, D], F16, kind="ExternalInput")
    lam_d = nc.dram_tensor("lam", [n_heads, 1], F32, kind="ExternalInput")
    out_d = nc.dram_tensor("out", [n_heads, s, D], F32, kind="ExternalOutput")

    with tile.TileContext(nc) as tc:
        with (
            tc.tile_pool(name="consts", bufs=1) as consts,
            tc.tile_pool(name="qk", bufs=2) as qk_pool,
            tc.tile_pool(name="vx", bufs=2) as vx_pool,
            tc.tile_pool(name="lamp", bufs=2) as lam_pool,
            tc.tile_pool(name="u", bufs=u_bufs) as u_pool,
            tc.tile_pool(name="ud", bufs=12) as ud_pool,
            tc.tile_pool(name="u32", bufs=3) as u32_pool,
            tc.tile_pool(name="o", bufs=1) as o_pool,
            tc.tile_pool(name="tr", bufs=8) as tr_pool,
            tc.tile_pool(name="w", bufs=3) as w_pool,
            tc.tile_pool(name="stats", bufs=3) as stats_pool,
            tc.tile_pool(name="small", bufs=8) as small_pool,
            tc.tile_pool(name="ps_sc", bufs=1, space="PSUM") as ps_scores,
            tc.tile_pool(name="ps_av", bufs=2, space="PSUM") as ps_av,
        ):
            eps_ap = consts.tile([128, 1], F32)
            nc.vector.memset(eps_ap, EPS)
            # prefetch the exp table set while the first DMAs run
            warm = consts.tile([128, 1], F32)
            nc.scalar.activation(warm, eps_ap, AF.Exp)

            # Two static sets of O^T staging tiles (heads alternate) so the
            # xbar-transpose pad rows 67..79 are zeroed exactly once.
            o_static = []
            for par in range(2):
                o1s = o_pool.tile([OROWS, s], F16, tag=f"o1_{par}")
                o2s = o_pool.tile([OROWS, s], F16, tag=f"o2_{par}")
                nc.gpsimd.memset(o1s[64:OROWS, :], 0.0)
                nc.gpsimd.memset(o2s[64:OROWS, :], 0.0)
                o_static.append((o1s, o2s))

            head_state = {}
            pending_ops = []

            def load_head(h):
                qt_sb = qk_pool.tile([2 * D, s], F16, tag="qt")
                kt_sb = qk_pool.tile([2 * D, s], F16, tag="kt")
                if h == 0:
                    # minimal first slices so group 0 can start ~2us in
                    nc.sync.dma_start(out=kt_sb[:, 0:256], in_=kt_d[h][:, 0:256])
                    nc.sync.dma_start(out=qt_sb[:, 0:QC], in_=qt_d[h][:, 0:QC])
                    nc.sync.dma_start(out=kt_sb[:, 256:s], in_=kt_d[h][:, 256:s])
                    nc.sync.dma_start(out=qt_sb[:, QC:s], in_=qt_d[h][:, QC:s])
                else:
                    nc.sync.dma_start(out=qt_sb, in_=qt_d[h])
                    nc.sync.dma_start(out=kt_sb, in_=kt_d[h])
                # AV weights: [0, V0..61, 0, V62, V63, ones] per k tile
                vx = vx_pool.tile([128, nkt, MAV], F16, tag="vx")
                vsrc = v_d[h].rearrange("(t p) d -> p t d", p=128)
                nc.sync.dma_start(out=vx[:, :, 1:63], in_=vsrc[:, :, 0:62])
                nc.sync.dma_start(out=vx[:, :, 64:66], in_=vsrc[:, :, 62:64])
                nc.vector.memset(vx[:, :, 0:1], 0.0)
                nc.vector.memset(vx[:, :, 63:64], 0.0)
                nc.vector.memset(vx[:, :, RROW : RROW + 1], 1.0)
                lamneg = lam_pool.tile([128, 1], F32, tag="lam")
                nc.sync.dma_start(out=lamneg, in_=lam_d[h].to_broadcast((128, 1)))
                nc.vector.tensor_scalar_mul(lamneg, lamneg, -1.0)
                o1, o2 = o_static[h % 2]
                w_head = w_pool.tile([128, nqt, D], F32, tag="w")
                head_state[h] = dict(qt=qt_sb, kt=kt_sb, vx=vx, lamneg=lamneg,
                                     o1=o1, o2=o2, w=w_head, mv=None)

            def score_group(h, qc, g):
                """One group of score units -> one PSUM tile -> exp -> U.
                Unit j = (kt = j//2, half = j%2). ScalarE ACTIVATE(Exp)
                drains slots 0..2 of 4-bank tiles and both slots of 2-bank
                tiles; slot 3 of each 4-bank tile goes to the DVE as a
                fp32 copy to SBUF + int16 Schraudolph tensor_scalar."""
                st = head_state[h]
                qt_sb, kt_sb = st["qt"], st["kt"]
                j0, n = gstarts[g], gsizes[g]
                ps = ps_scores.tile(
                    [128, n * QC], F32, tag=("ps4" if n == 4 else "ps2"))
                for i in range(n):
                    j = j0 + i
                    kt = j // 2
                    rb = (j % 4) * D2
                    dsl = slice(rb, rb + D2)
                    nc.tensor.matmul(
                        ps[:, i * QC : (i + 1) * QC],
                        kt_sb[dsl, kt * KTILE : (kt + 1) * KTILE],
                        qt_sb[dsl, qc * QC : (qc + 1) * QC],
                        start=True, stop=True,
                        tile_position=(rb, 0),
                    )
                na = n - 1 if (DVE_SPLIT and n == 4) else n
                u = u_pool.tile([128, (3 if DVE_SPLIT else 4) * QC], F16, tag="u")
                nc.scalar.activation(
                    u[:, 0 : na * QC], ps[:, 0 : na * QC], AF.Exp,
                    scale=SCALE)
                udve = None
                if na < n:
                    t32 = u32_pool.tile([128, QC], F32, tag="u32")
                    nc.vector.tensor_copy(t32, ps[:, 3 * QC : 4 * QC])
                    udve = ud_pool.tile([128, QC], I16, tag="ud")
                    nc.vector.tensor_scalar(
                        out=udve, in0=t32,
                        scalar1=EXP_A, scalar2=EXP_B,
                        op0=ALU.mult, op1=ALU.add)
                st[("us", qc)].append((u, udve))

            def av_chunk(h, qc, mlist):
                """AV matmuls m in mlist; m = half*nkt + kt."""
                st = head_state[h]
                vx = st["vx"]
                us = st[("us", qc)]
                for m in mlist:
                    half, kt = m // nkt, m % nkt
                    if kt == 0:
                        pav_new = ps_av.tile([MAV, QC], F32, tag="pav")
                        st[("pav", qc, half)] = pav_new
                    pav = st[("pav", qc, half)]
                    g, slot = unit_slot(kt * 2 + half)
                    ua, udve = us[g]
                    if udve is not None and slot == 3:
                        u_ap = udve[:, :].bitcast(F16)
                    else:
                        u_ap = ua[:, slot * QC : (slot + 1) * QC]
                    nc.tensor.matmul(
                        pav, vx[:, kt, :], u_ap,
                        start=(kt == 0), stop=(kt == nkt - 1),
                    )
                    if kt == nkt - 1:
                        o_sb = st["o2"] if half else st["o1"]
                        nc.vector.tensor_copy(
                            o_sb[0:MAV, qc * QC : (qc + 1) * QC], pav)
                        del st[("pav", qc, half)]

            def norm_chunk(h, qc):
                """DMA-xbar transpose of one q chunk; the DVE combine+stats
                ops go through the pump so they interleave with the next
                step's exp work instead of blocking the ps2 drains.
                tr layout r-index: 1..62 = V0..61, 64,65 = V62,63, 66 = r."""
                st = head_state[h]
                o1, o2, lamneg = st["o1"], st["o2"], st["lamneg"]
                if st["mv"] is None:
                    mv_new = stats_pool.tile([128, nqt, 2], F32, tag="mv")
                    st["mv"] = mv_new
                mv = st["mv"]
                csl = slice(qc * QC, (qc + 1) * QC)
                tr1 = tr_pool.tile([128, tpq, OROWS], F16, tag="tr1")
                nc.sync.dma_start_transpose(tr1, o1[:, csl])
                tr2 = tr_pool.tile([128, tpq, OROWS], F16, tag="tr2")
                nc.sync.dma_start_transpose(tr2, o2[:, csl])
                r1v = small_pool.tile([128, tpq], F32, tag="r1v")
                r2v = small_pool.tile([128, tpq], F32, tag="r2v")
                w2 = small_pool.tile([128, tpq, D], F32, tag="w2")
                s6 = small_pool.tile([128, tpq, 6], F32, tag="s6")
                w = st["w"][:, qc * tpq : (qc + 1) * tpq, :]
                ops = [
                    lambda: nc.vector.reciprocal(r1v, tr1[:, :, RROW : RROW + 1]),
                    lambda: nc.vector.reciprocal(r2v, tr2[:, :, RROW : RROW + 1]),
                    lambda: nc.vector.tensor_scalar_mul(r2v, r2v, lamneg),
                ]
                for lo, hi, tlo in ((0, 62, 1), (62, 64, 64)):
                    nw = hi - lo
                    r1b = r1v[:][:, :, None].broadcast_to((128, tpq, nw))
                    r2b = r2v[:][:, :, None].broadcast_to((128, tpq, nw))
                    ops += [
                        lambda lo=lo, hi=hi, tlo=tlo, nw=nw, r1b=r1b:
                            nc.vector.tensor_mul(
                                w[:, :, lo:hi], tr1[:, :, tlo : tlo + nw], r1b),
                        lambda lo=lo, hi=hi, tlo=tlo, nw=nw, r2b=r2b:
                            nc.vector.tensor_mul(
                                w2[:, :, lo:hi], tr2[:, :, tlo : tlo + nw], r2b),
                        lambda lo=lo, hi=hi:
                            nc.vector.tensor_add(
                                w[:, :, lo:hi], w[:, :, lo:hi], w2[:, :, lo:hi]),
                    ]
                for t in range(tpq):
                    ops += [
                        lambda t=t: nc.vector.bn_stats(
                            out=s6[:, t, :], in_=w[:, t, :]),
                        lambda t=t: nc.vector.bn_aggr(
                            out=mv[:, qc * tpq + t, :], in_=s6[:, t, :]),
                    ]
                return ops

            def finish_head_a(h):
                """rstd = (1-lam0)*rsqrt(var+eps): Schraudolph ln seed ->
                fp16 Schraudolph exp -> 2 Newton steps (last folds the
                (1-lam0) factor). Ops dribbled between score groups."""
                st = head_state[h]
                mv = st["mv"]
                x = stats_pool.tile([128, nqt], F32, tag="x")
                sd = stats_pool.tile([128, nqt], F32, tag="sd")
                y16 = stats_pool.tile([128, nqt], F16, tag="y16")
                y = stats_pool.tile([128, nqt], F32, tag="y")
                t = stats_pool.tile([128, nqt], F32, tag="t")
                st["y"] = y
                c = 1.0 - LAMBDA_INIT
                ops = [
                    lambda: nc.vector.tensor_scalar_add(x, mv[:, :, 1], EPS),
                    lambda: nc.vector.tensor_scalar(
                        out=sd, in0=x[:].bitcast(I32),
                        scalar1=RS_K1, scalar2=RS_K2,
                        op0=ALU.mult, op1=ALU.add),
                    lambda: nc.vector.tensor_scalar(
                        out=y16[:].bitcast(I16), in0=sd,
                        scalar1=RS_A, scalar2=RS_B,
                        op0=ALU.mult, op1=ALU.add),
                    # Newton 1: y = y16*(1.5 - 0.5*x*y16^2)
                    lambda: nc.vector.tensor_mul(t, y16, y16),
                    lambda: nc.vector.tensor_mul(t, t, x),
                    lambda: nc.vector.tensor_scalar(
                        out=t, in0=t, scalar1=-0.5, scalar2=1.5,
                        op0=ALU.mult, op1=ALU.add),
                    lambda: nc.vector.tensor_mul(y, y16, t),
                    # Newton 2 with (1-lam0) folded in
                    lambda: nc.vector.tensor_mul(t, y, y),
                    lambda: nc.vector.tensor_mul(t, t, x),
                    lambda: nc.vector.tensor_scalar(
                        out=t, in0=t, scalar1=-0.5 * c, scalar2=1.5 * c,
                        op0=ALU.mult, op1=ALU.add),
                    lambda: nc.vector.tensor_mul(y, y, t),
                ]
                pending_ops.extend(ops)

            def finish_head_b(h):
                st = head_state[h]
                mv, y = st["mv"], st["y"]
                m2 = stats_pool.tile([128, nqt], F32, tag="m2")
                w3 = st["w"][:, :, :]
                yb = y[:][:, :, None].broadcast_to((128, nqt, D))
                m2b = m2[:][:, :, None].broadcast_to((128, nqt, D))
                wd = out_d[h].rearrange("(t p) d -> p t d", p=128)
                pending_ops.extend([
                    lambda: nc.vector.tensor_mul(m2, mv[:, :, 0], y),
                    lambda: nc.vector.tensor_mul(w3, w3, yb),
                    lambda: nc.vector.tensor_sub(w3, w3, m2b),
                    lambda: nc.gpsimd.dma_start(out=wd, in_=w3),
                    lambda: head_state.pop(h),
                ])

            def pump(n):
                for _ in range(min(n, len(pending_ops))):
                    pending_ops.pop(0)()

            # ---- emission: one flat (head, chunk) pipeline; scores of step
            # s+1 interleave with AV of step s across head boundaries ----
            load_head(0)
            deferred_norm = []
            nsteps = n_heads * nq
            for step in range(nsteps + 1):
                if step < nsteps:
                    h, qc = divmod(step, nq)
                    head_state[h][("us", qc)] = []
                else:
                    h = qc = None
                ph, pqc = divmod(step - 1, nq)
                if step == nsteps:
                    for gl in range(ngrp - 6, ngrp):
                        av_chunk(ph, pqc,
                                 range(gstarts[gl], gstarts[gl] + gsizes[gl]))
                last = step == nsteps - 1
                for g in range(ngrp + 1):
                    if step < nsteps and g < ngrp:
                        score_group(h, qc, g)
                    if step > 0 and g > 0 and step < nsteps:
                        av_chunk(ph, pqc,
                                 range(gstarts[g - 1],
                                       gstarts[g - 1] + gsizes[g - 1]))
                    if last and g >= 7:
                        # final chunk: start its AV early (7 groups behind
                        # its scores) so the tail after the last exp shrinks
                        gl = g - 7
                        av_chunk(h, qc,
                                 range(gstarts[gl], gstarts[gl] + gsizes[gl]))
                    if g >= 3 or step == nsteps:
                        pump(4)
                if step > 0:
                    head_state[ph].pop(("us", pqc))
                    # norm DVE ops deferred one extra step so the transposes
                    # are long done before the pumped ops reach the queue
                    pending_ops.extend(deferred_norm)
                    deferred_norm = norm_chunk(ph, pqc)
                    if pqc == min(1, nq - 1) and ph > 0:
                        finish_head_a(ph - 1)
                    if pqc == min(2, nq - 1) and ph > 0:
                        finish_head_b(ph - 1)
                if step < nsteps and qc == nq - 2 and h + 1 < n_heads:
                    load_head(h + 1)  # prefetch next head's tensors
            pending_ops.extend(deferred_norm)
            finish_head_a(n_heads - 1)
            finish_head_b(n_heads - 1)
            pump(len(pending_ops))

    nc.compile()
    return nc


_PROGRAM_CACHE = {}


def _get_program():
    key = (HEADS_PER_CORE, S)
    if key not in _PROGRAM_CACHE:
        _PROGRAM_CACHE[key] = build_program()
    return _PROGRAM_CACHE[key]


def shard_inputs(query, key, value, lambda_params):
    """Full [B,H,S,D] inputs -> per-core input maps (host-side prep)."""
    q = np.asarray(query, dtype=np.float32).reshape(B * H, S, D)
    k = np.asarray(key, dtype=np.float32).reshape(B * H, S, D)
    v = np.asarray(value, dtype=np.float32).reshape(B * H, S, D)
    lam = np.asarray(lambda_params, dtype=np.float32)
    lam_full = np.tile(lam, B)  # pair i = (b=i//H, h=i%H) -> lambda[i%H]
    in_maps = []
    for c in range(N_CORES):
        sl = slice(c * HEADS_PER_CORE, (c + 1) * HEADS_PER_CORE)
        qt = q[sl].transpose(0, 2, 1).astype(np.float16)
        kt = k[sl].transpose(0, 2, 1).astype(np.float16)
        in_maps.append({
            "qt": np.ascontiguousarray(np.concatenate([qt, qt], axis=1)),
            "kt": np.ascontiguousarray(np.concatenate([kt, kt], axis=1)),
            "v": np.ascontiguousarray(v[sl]).astype(np.float16),
            "lam": np.ascontiguousarray(lam_full[sl].reshape(-1, 1)),
        })
    return in_maps


def kernel(query, key, value, lambda_params, trace=False):
    nc = _get_program()
    in_maps = shard_inputs(query, key, value, lambda_params)
    res = run_bass_kernel_spmd(nc, in_maps, core_ids=list(range(N_CORES)),
                               trace=trace)
    out = np.concatenate([r["out"] for r in res.results], axis=0)
    out = out.reshape(B, H, S, D).astype(np.float32)
    if trace:
        kernel.last_exec_time_ns = res.exec_time_ns
        kernel.last_results = res
    return out
